# revision 54
# baseline (speedup 1.0000x reference)
"""Biaffine edge attention on 8 Trainium2 NeuronCores (fp16, PE-roofline schedule).

Math (per batch b):
    out[i,o] = head[i,:] @ U @ dep[o,:] + head[i,:]@wh + dep[o,:]@wd + b
with head/dep [S=2048, D=256], U [D,D], edge_W = [wh | wd] (each [D]).

Sharding: pure data-parallel over batch B=8 -> one batch per core,
U / edge_W / edge_b replicated. No collectives.

Host prep (layout only + the tiny rank-1 bias):
    headT/depT: inputs pre-transposed to [D, S] fp16 and packed
        [dc0 | dc1] side-by-side as [128, 2S] -- the PE needs the
        contraction dim on partitions for both operands, host-side
        layout beats 64 PE transposes, and the single-DMA packing gives
        8KB DRAM elements (~340 GB/s vs ~220 at 2KB).
    hs2[p, j] = head[j*128+p, :] @ wh + b   (per-row bias, [128, 16] f32)
    u2 = [U[:128, :] | U[128:, :]]          ([128, 512] fp16)
    wd2[p, eb] = wd[eb*128+p]               ([128, 2] f32)

Per-core kernel (fp16 matmuls, f32 PSUM, fp16 stores upcast on host):
    ATf[e,i] = sum_d U[d,e] headT[d,i] + wd[e]    (ds[o] rides the
               e-contraction of the out matmul for free)
    out[i,o] = sum_e ATf[e,i] depT[e,o] + hs2[i]  (bias fused in the
               PSUM->SBUF eviction on ACT/DVE)

Schedule notes (exec ~49.2-51.8us on HW; PE-roofline: 27.6us out
matmuls + 3.5us ATf + ~7.5us framework preamble + ~4us sem-reset
teardown; +-1.7us run variance from the free-running HAM window):
  - the 8 big input chunks are FIFO-serialized on the ACT HWDGE ring
    in priority order (head, then dep) -- parallel rings
    packet-interleave on the shared SDMA engines and delay the head
    bytes the ATf phase waits on.  The tiny consts (u2/wd2/hs2) ride
    the SP ring, which is otherwise idle until the stores begin.
    Chunk-major DRAM packing makes every load a sequential stream
    (strided loads measured ~195-220 GB/s vs ~300+ contiguous).
  - 6 junk matmuls on a memset tile warm the PE HAM clock gate
    (1.2 -> 2.4 GHz needs ~3.4us of sustained PE-busy) during the
    loads; any >300ns PE gap before the flip postpones it a window.
  - matmuls stream N=512 (ISA max moving dim) but land pairwise in
    2-bank [128,1024] PSUM tiles: one DVE + one ACT 1024-wide
    eviction per row-block halves the eviction instruction count.
    PSUM = 4 tiles x 2 banks = 2 row-blocks in flight.
  - row bias (hs2) and ATf bias (wd2) ride the evictions for free;
    ds rides the e-contraction; hs2 is a host-precomputed input (the
    on-device gpsimd-mul + DVE-reduce chain was a 13us serial stall
    in the original schedule).
  - rows 0-1 and 15 split in halves: the early h=0 halves only need
    the first dep half (fills the PE window while the dep tail is on
    the DMA ring); row 15's halves make the final store small.
  - stores: one [128,2048] fp16 DMA per row-block on the SP ring.
"""

import numpy as np

import concourse.bass as bass
import concourse.tile as tile
from concourse import bacc, mybir
from concourse.bass_utils import run_bass_kernel_spmd

B, S, D = 8, 2048, 256
P = 128          # partitions
NB = 1024        # matmul moving free-dim block (two PSUM banks of fp32)
NI = S // P      # 16 output row blocks
ND = D // P      # 2 contraction chunks
NWARM = 3        # PE warm-up filler matmuls (cover load latency, warm HAM)
F32 = mybir.dt.float32
F16 = mybir.dt.float16

Ident = mybir.ActivationFunctionType.Identity


def build_nc(reps=1):
    """reps>1 wraps the body in a HW For_i loop -- used only for timing."""
    nc = bacc.Bacc("TRN2", target_bir_lowering=False, debug=False, num_devices=B)

    # chunk-major [4, 128, 1024]: chunk j = (h, dc) is a fully contiguous
    # 256KB block, so each load DMA is a sequential DRAM stream (line rate)
    headT_d = nc.dram_tensor("headT", [4 * ND, P, S // 4], F16,
                             kind="ExternalInput")
    depT_d = nc.dram_tensor("depT", [2 * ND, P, S // 2], F16,
                            kind="ExternalInput")
    u2_d = nc.dram_tensor("u2", [P, ND * D], F16, kind="ExternalInput")
    wd2_d = nc.dram_tensor("wd2", [P, ND], F32, kind="ExternalInput")
    hs2_d = nc.dram_tensor("hs2", [P, NI], F32, kind="ExternalInput")
    out_d = nc.dram_tensor("out", [S, S], F16, kind="ExternalOutput")

    with tile.TileContext(nc) as tc:
        with (
            tc.tile_pool(name="const", bufs=1) as cpool,
            tc.tile_pool(name="persist", bufs=1) as ppool,
            tc.tile_pool(name="outbuf", bufs=3) as outbuf,
            tc.tile_pool(name="hbuf", bufs=2) as hbuf,
            tc.tile_pool(name="ps", bufs=4, space=bass.MemorySpace.PSUM) as ps,
        ):
            def body():
                # ---- loads: priority order on the ACT HWDGE ring.
                # The DRAM packing interleaves the dc chunks at half-S
                # granularity: [dc0 h0 | dc1 h0 | dc0 h1 | dc1 h1], so each
                # 512KB transfer (4KB DRAM elements, near line rate)
                # unlocks the next compute stage: head-h0 -> ATf ic0,
                # head-h1 -> ic1, dep-h0 -> boundary halves, dep-h1 ->
                # full rows. ----
                # Consts ride the SP ring (idle until the stores begin) so
                # the ACT ring starts streaming head bytes immediately;
                # u2 is tiny and lands before the first head chunk does.
                # (Putting ANY big chunk on a second ring loses: even two
                # concurrent transfers packet-interleave on the shared
                # SDMA engines and push the critical receipt later.)
                u2 = cpool.tile([P, ND * D], F16, name="u2", tag="u2")
                nc.sync.dma_start(u2[:], u2_d[:])
                wd2 = cpool.tile([P, ND], F32, name="wd2", tag="wd2")
                nc.sync.dma_start(wd2[:], wd2_d[:])
                hs2 = cpool.tile([P, NI], F32, name="hs2", tag="hs2")
                nc.sync.dma_start(hs2[:], hs2_d[:])
                headT2 = ppool.tile([P, ND * S], F16, name="headT2",
                                    tag="headT2")
                depT2 = ppool.tile([P, ND * S], F16, name="depT2",
                                   tag="depT2")
                for j in range(4 * ND):
                    nc.scalar.dma_start(
                        headT2[:, j * 512:(j + 1) * 512], headT_d[j])
                for j in range(2 * ND):
                    nc.scalar.dma_start(
                        depT2[:, j * NB:(j + 1) * NB], depT_d[j])

                def hcol(dc, ic, k):
                    # head col of 512-block k within i-chunk ic, d-chunk dc
                    return ic * 2 * NB + dc * NB + k * 512

                def dcol(eb, c):
                    # dep col of 512-wide o-chunk c, e-chunk eb
                    return (c // 2) * 2 * NB + eb * NB + (c % 2) * 512

                # ---- PE warm-up fillers on a memset tile (no DMA deps;
                #      PE->PE WAW on pool tiles is program-order-free) ----
                warm = cpool.tile([P, 512], F16, name="warm", tag="warm")
                nc.vector.memset(warm[:], 0.0)
                for _ in range(NWARM):
                    pw = ps.tile([P, NB], F32, name="ps", tag="ps")
                    nc.tensor.matmul(pw[:, 0:512], warm[:, 0:P],
                                     warm[:], start=True, stop=True)

                # ---- ATf[e, i] = U^T @ headT + wd (bias in eviction) ----
                atf = [ppool.tile([P, S], F16, name=f"atf{eb}", tag=f"atf{eb}")
                       for eb in range(ND)]

                def atf_chunk(ic):
                    # one [128,1024] 2-bank PSUM tile per (ic, eb), filled
                    # by 2x2 N=512 matmuls, drained by ONE 1024-wide
                    # eviction (ISA caps the moving dim at 512).
                    for eb in range(ND):
                        pa = ps.tile([P, NB], F32, name="ps", tag="ps")
                        for dc in range(ND):
                            for k in range(2):
                                nc.tensor.matmul(
                                    pa[:, k * 512:(k + 1) * 512],
                                    u2[:, dc * D + eb * P:
                                       dc * D + (eb + 1) * P],
                                    headT2[:, hcol(dc, ic, k):
                                           hcol(dc, ic, k) + 512],
                                    start=(dc == 0), stop=(dc == ND - 1),
                                )
                        dst = atf[eb][:, ic * NB:(ic + 1) * NB]
                        if eb == 0:
                            nc.vector.tensor_scalar_add(
                                dst, pa[:], wd2[:, eb:eb + 1])
                        else:
                            nc.scalar.activation(
                                dst, pa[:], Ident, bias=wd2[:, eb:eb + 1])

                # out row-block: 2 eb x 4 N=512 matmuls into two 2-bank
                # PSUM tiles; one DVE + one ACT 1024-wide eviction per row.
                def out_row(ib):
                    ot = outbuf.tile([P, S], F16, name="ot", tag="ot")
                    pos = [ps.tile([P, NB], F32, name="ps", tag="ps")
                           for _ in range(2)]
                    for eb in range(ND):
                        for c in range(4):
                            nc.tensor.matmul(
                                pos[c // 2][:, (c % 2) * 512:
                                            (c % 2 + 1) * 512],
                                atf[eb][:, ib * P:(ib + 1) * P],
                                depT2[:, dcol(eb, c):dcol(eb, c) + 512],
                                start=(eb == 0), stop=(eb == ND - 1),
                            )
                    for h in range(2):
                        dst = ot[:, h * NB:(h + 1) * NB]
                        if h == 0:
                            nc.vector.tensor_scalar_add(
                                dst, pos[h][:], hs2[:, ib:ib + 1])
                        else:
                            nc.scalar.activation(
                                dst, pos[h][:], Ident, bias=hs2[:, ib:ib + 1])
                    nc.sync.dma_start(out_d[ib * P:(ib + 1) * P, :], ot[:])

                # Half-row-block for the load-boundary rows: h=0 halves
                # only need the first dep half; h=1 halves run mid-stream.
                def out_half(ib, h, split_evict=False):
                    ot = hbuf.tile([P, NB], F16, name="oth", tag="oth")
                    po = ps.tile([P, NB], F32, name="ps", tag="ps")
                    for eb in range(ND):
                        for k in range(2):
                            c = 2 * h + k
                            nc.tensor.matmul(
                                po[:, k * 512:(k + 1) * 512],
                                atf[eb][:, ib * P:(ib + 1) * P],
                                depT2[:, dcol(eb, c):dcol(eb, c) + 512],
                                start=(eb == 0), stop=(eb == ND - 1),
                            )
                    if split_evict:
                        # both engines in parallel -- used for the final
                        # halves so the closing store starts sooner
                        nc.vector.tensor_scalar_add(
                            ot[:, 0:512], po[:, 0:512], hs2[:, ib:ib + 1])
                        nc.scalar.activation(
                            ot[:, 512:NB], po[:, 512:NB], Ident,
                            bias=hs2[:, ib:ib + 1])
                    elif (ib + h) % 2 == 0:
                        nc.vector.tensor_scalar_add(
                            ot[:], po[:], hs2[:, ib:ib + 1])
                    else:
                        nc.scalar.activation(
                            ot[:], po[:], Ident, bias=hs2[:, ib:ib + 1])
                    nc.sync.dma_start(
                        out_d[ib * P:(ib + 1) * P, h * NB:(h + 1) * NB],
                        ot[:])

                # rows 0-1 split at the dep-h0/h1 load boundary (their h=0
                # halves also absorb dep-receipt jitter before the full
                # rows need all of dep); the last row is split so the
                # final store is small and its evictions run on both
                # engines in parallel.
                atf_chunk(0)
                atf_chunk(1)
                out_half(0, 0)
                out_half(1, 0)
                for ib in range(2, NI - 1):
                    out_row(ib)
                    if ib == 8:
                        out_half(0, 1)
                    elif ib == 10:
                        out_half(1, 1)
                out_half(NI - 1, 0, split_evict=True)
                out_half(NI - 1, 1, split_evict=True)

            if reps > 1:
                with tc.For_i(0, reps, 1):
                    body()
            else:
                body()

    nc.finalize()
    return nc


_NC_CACHE = {}


def _get_nc(reps=1):
    if reps not in _NC_CACHE:
        _NC_CACHE[reps] = build_nc(reps)
    return _NC_CACHE[reps]


def make_in_maps(head, dep, edge_U, edge_W, edge_b):
    head = np.asarray(head, np.float32)
    dep = np.asarray(dep, np.float32)
    # [B, D, S] -> chunk-major [B, 4, 128, 1024]: chunk j=(h, dc) holds
    # i-cols of half h for d-chunk dc, each chunk contiguous in DRAM
    def pack(x):
        t = x.astype(np.float16).transpose(0, 2, 1)     # [B, D, S]
        t = t.reshape(B, ND, P, 2, S // 2)              # dc, p, h, s
        t = t.transpose(0, 3, 1, 2, 4)                  # h, dc, p, s
        return np.ascontiguousarray(t.reshape(B, 2 * ND, P, S // 2))

    def pack_q(x):
        t = x.astype(np.float16).transpose(0, 2, 1)     # [B, D, S]
        t = t.reshape(B, ND, P, 2, 2, S // 4)           # dc, p, h, k, s
        t = t.transpose(0, 3, 1, 4, 2, 5)               # h, dc, k, p, s
        return np.ascontiguousarray(t.reshape(B, 4 * ND, P, S // 4))

    headT = pack_q(head)
    depT = pack(dep)
    u = np.asarray(edge_U, np.float32).astype(np.float16)
    u2 = np.ascontiguousarray(
        np.concatenate([u[dc * P:(dc + 1) * P, :] for dc in range(ND)],
                       axis=1))                             # [128, 512]
    w = np.asarray(edge_W, np.float32).reshape(-1)
    wh, wd = w[:D], w[D:]
    wd2 = np.ascontiguousarray(wd.reshape(ND, P).T.astype(np.float32))
    b0 = float(np.asarray(edge_b, np.float32).reshape(-1)[0])
    hs = head @ wh + b0                                     # [B, S] f32
    hs2 = np.ascontiguousarray(
        hs.reshape(B, NI, P).transpose(0, 2, 1))            # [B, 128, 16]
    return [
        {"headT": headT[b], "depT": depT[b], "u2": u2, "wd2": wd2,
         "hs2": hs2[b]}
        for b in range(B)
    ]


def kernel(head, dep, edge_U, edge_W, edge_b):
    nc = _get_nc()
    in_maps = make_in_maps(head, dep, edge_U, edge_W, edge_b)
    last_err = None
    for _ in range(3):  # transient device errors happen on this shared env
        try:
            res = run_bass_kernel_spmd(nc, in_maps, core_ids=list(range(B)))
            break
        except Exception as e:  # noqa: BLE001
            last_err = e
    else:
        raise last_err
    return np.stack(
        [res.results[b]["out"].astype(np.float32) for b in range(B)], axis=0)
